# revision 53
# baseline (speedup 1.0000x reference)
"""CustomGaussianLayer Trainium2 kernel.

Math: out[b,o] = sum_{i,g} exp(-0.5*((tanh(x[b,i])-c_g)/w)^2) * coeff[o,i,g]*W[o,i]
 == E @ W2T  with  E[b, k=(g,i)] Gaussian basis,  W2T[k, o] folded weights.

Factored basis:  exp(-a*(t-c)^2) = exp(-a*t^2) * exp(2ac*t) * exp(-a*c^2), a=24.5;
the exp(-a*c^2) factor is folded into the weights host-side.  Center spacing is
uniform (2/7), so B_{g+1} = B_g * rho with rho = exp(14*t): only seeds g=0,4 use
ACT exp; the rest chain on DVE bf16 multiplies (2x mode).  Everything that moves
(w2, E, out, x) is bf16; matmul bf16 @ 1 col/cycle; accumulate fp32 psum.
Per core (data-parallel over batch, 1024 rows): 256 matmuls [128k,128o,512b].
"""

import numpy as np
import ml_dtypes

import concourse.bacc as bacc
import concourse.bass as bass
import concourse.mybir as mybir
import concourse.tile as tile
from concourse.bass_utils import run_bass_kernel_spmd
from concourse.tile import add_dep_helper

G = 8
I_SZ = 512
O_SZ = 512
B = 8192
NCORES = 8
B_SH = B // NCORES          # 1024 batch rows per core
K = I_SZ * G                # 4096 contraction
N_IBLK = I_SZ // 128        # 4 partition blocks of i
FREE = N_IBLK * B_SH        # 4096 free layout (i_blk, b)
HALF = FREE // 2            # 2048 (i_blk 0-1 | 2-3)
N_OT = O_SZ // 128          # 4 output tiles
N_BC = B_SH // 512          # 2 batch chunks of 512 (psum free limit fp32)
N_KT = K // 128             # 32 k-tiles

ALPHA = 24.5
RHO_SCALE = 2.0 * ALPHA * (2.0 / (G - 1))   # 14.0 = exp-ratio between centers
N_WARMUP = 8
CENTERS = np.linspace(-1.0, 1.0, G).astype(np.float64)
SEED_GS = (0, 4)

F32 = mybir.dt.float32
BF16 = mybir.dt.bfloat16
AF = mybir.ActivationFunctionType
ALU = mybir.AluOpType

_NC_CACHE = {}


def build_nc():
    nc = bacc.Bacc("TRN2", target_bir_lowering=False)
    xt_d = nc.dram_tensor("xt", [I_SZ, B_SH], BF16, kind="ExternalInput")
    w2t_d = nc.dram_tensor("w2t", [K, O_SZ], BF16, kind="ExternalInput")
    out_d = nc.dram_tensor("out_t", [O_SZ, B_SH], BF16, kind="ExternalOutput")

    with tile.TileContext(nc) as tc:
        with (
            tc.tile_pool(name="w2", bufs=1) as w2_pool,
            tc.tile_pool(name="xx", bufs=1) as xx_pool,
            tc.tile_pool(name="ee", bufs=1) as ee_pool,
            tc.tile_pool(name="ps", bufs=1, space="PSUM") as ps_pool,
        ):
            # xt q0 rides the SYNC ring (it starts ~0.4us earlier than the
            # scalar ring, and xt gates the whole tanh->E0 chain at ~10us);
            # w2 kt0 rides the scalar ring ahead of the ACT-table warmup (it
            # has ~2.7us of slack before its first matmul).  Each HWDGE ring
            # is a serial ~2us-per-DMA pipe at the head, so the two critical
            # payloads must ride separate rings.
            xt_sb = xx_pool.tile([128, FREE], BF16, tag="xt")
            w2_all = w2_pool.tile([128, N_KT * O_SZ], BF16, tag="w2all")
            w2t_v = w2t_d[:, :].rearrange("(kt p) o -> p kt o", p=128)
            xtq0_dma = nc.sync.dma_start(xt_sb[:, 0:512], xt_d[0:128, 0:512])
            w2kt0_dma = nc.scalar.dma_start(
                w2_all[:, 0:O_SZ].rearrange("p (kt o) -> p kt o", o=O_SZ),
                w2t_v[:, 0:1, :],
            )

            # trigger the ACT spline-table load immediately (costs ~1.3us;
            # otherwise it delays the first tanh)
            actwarm = xx_pool.tile([128, 1], F32, tag="actwarm")
            aw_i = nc.scalar.activation(
                actwarm[:], nc.const_aps.tensor(0.0, (128, 1)), AF.Exp,
            )
            add_dep_helper(aw_i.ins, w2kt0_dma.ins, sync=False,
                           reason="scalar queue: kt0 issue, then table load")

            # PE clock-ramp warmup: matmuls on a memset tile, gated on no DMA.
            # memset on gpsimd, whose queue starts earliest (~6.1us).
            wu = xx_pool.tile([128, 640], BF16, tag="wu")
            ms_i = nc.gpsimd.memset(wu[:], 0.0)

            tt = xx_pool.tile([128, FREE], F32, tag="tt")
            sq = xx_pool.tile([128, FREE], F32, tag="sq")
            a_t = xx_pool.tile([128, FREE], BF16, tag="aa")
            rho = xx_pool.tile([128, FREE], BF16, tag="rho")
            b0_t = xx_pool.tile([128, FREE], BF16, tag="b0")
            b4_t = xx_pool.tile([128, FREE], BF16, tag="b4")
            e_t = [
                [ee_pool.tile([128, HALF], BF16, name=f"e{h}_{g}",
                              tag=f"e{h}_{g}") for g in range(G)]
                for h in range(2)
            ]
            o_sb = xx_pool.tile([128, N_OT * N_BC * 512], BF16, tag="osb")
            psum = [
                [
                    ps_pool.tile([128, 512], F32, name=f"ps{ot}_{bc}",
                                 tag=f"ps{ot}_{bc}")
                    for bc in range(N_BC)
                ]
                for ot in range(N_OT)
            ]
            for w in range(N_WARMUP):
                nc.tensor.matmul(
                    psum[3][1][:], wu[:, 0:128], wu[:, 128:640],
                    start=(w == 0), stop=(w == N_WARMUP - 1),
                )

            # ---- remaining input DMAs ----
            # Later xt chunks ride the gpsimd SWDGE queue, where there is
            # slack; w2 owns the sync HWDGE ring with kt0 first.
            # w2 kt2-3 rides the gpsimd ring between the xt chunks: the sync
            # ring alone can't land one chunk per ladder-step at the head
            # (~2.1us/DMA completion pace), which cost a kt3 stall.
            xt_dmas = [
                nc.gpsimd.dma_start(xt_sb[:, 512:1024], xt_d[0:128, 512:1024]),
                nc.gpsimd.dma_start(
                    w2_all[:, 2 * O_SZ:4 * O_SZ]
                    .rearrange("p (kt o) -> p kt o", o=O_SZ),
                    w2t_v[:, 2:4, :],
                ),
                nc.gpsimd.dma_start(xt_sb[:, 1024:2048], xt_d[128:256, :]),
                nc.gpsimd.dma_start(
                    xt_sb[:, 2048:4096].rearrange("p (ib b) -> p ib b", b=B_SH),
                    xt_d[256:512, :].rearrange("(ib p) b -> p ib b", p=128),
                ),
            ]
            add_dep_helper(xt_dmas[0].ins, ms_i.ins, sync=False,
                           reason="wu memset first on gpsimd")
            for i in range(1, len(xt_dmas)):
                add_dep_helper(xt_dmas[i].ins, xt_dmas[i - 1].ins, sync=False,
                               reason="xt DMA order")

            def w2_dma(kt_lo, kt_hi):
                return nc.sync.dma_start(
                    w2_all[:, kt_lo * O_SZ:kt_hi * O_SZ]
                    .rearrange("p (kt o) -> p kt o", o=O_SZ),
                    w2t_v[:, kt_lo:kt_hi, :],
                )

            # chunk sizes matched to PE consumption (ladder pace ~0.86us/kt)
            # vs the sync HWDGE ring's ~0.65us issue + ~2.7us land latency:
            # small chunks early so kt_i lands before its first matmul.
            # (kt0 via scalar ring, kt2-3 via the gpsimd ring above)
            w2_dmas = [w2_dma(1, 2), w2_dma(4, 8),
                       w2_dma(8, 16), w2_dma(16, 24), w2_dma(24, 32)]
            add_dep_helper(w2_dmas[0].ins, xtq0_dma.ins, sync=False,
                           reason="sync queue: xt q0 first")
            for i in range(1, len(w2_dmas)):
                add_dep_helper(w2_dmas[i].ins, w2_dmas[i - 1].ins, sync=False,
                               reason="w2 DMA consumer order")

            # ---- basis production ----
            # h0 in chunks (512,512,1024) to cut startup latency; h1 coarse.
            act_chain = [aw_i]
            gps_chain = [xt_dmas[-1]]   # GPSIMD queue: memset, xt DMAs, sq
            dve_chain = []              # DVE queue: E muls, drain copies

            def act(ins_f):
                i = ins_f()
                add_dep_helper(i.ins, act_chain[-1].ins, sync=False,
                               reason="ACT order")
                act_chain.append(i)
                return i

            def gps(ins_f):
                i = ins_f()
                if gps_chain:
                    add_dep_helper(i.ins, gps_chain[-1].ins, sync=False,
                                   reason="GPSIMD order")
                gps_chain.append(i)
                return i

            def dve(ins_f):
                i = ins_f()
                if dve_chain:
                    add_dep_helper(i.ins, dve_chain[-1].ins, sync=False,
                                   reason="DVE order")
                dve_chain.append(i)
                return i

            for h in range(2):
                hb = h * HALF
                parts = [(0, 512), (512, 1024), (1024, 2048)] \
                    if h == 0 else [(0, HALF)]
                for pi, (lo, hi) in enumerate(parts):
                    s = slice(hb + lo, hb + hi)
                    act(lambda: nc.scalar.activation(tt[:, s], xt_sb[:, s],
                                                     AF.Tanh))
                    act(lambda: nc.scalar.activation(
                        b0_t[:, s], tt[:, s], AF.Exp,
                        scale=float(2.0 * ALPHA * CENTERS[0])))
                    # sq for the very first chunk on DVE (shortest critical
                    # path); later chunks on the otherwise-idle gpsimd
                    if h == 0 and pi == 0:
                        dve(lambda: nc.vector.tensor_tensor(
                            sq[:, s], tt[:, s], tt[:, s], op=ALU.mult))
                    else:
                        gps(lambda: nc.gpsimd.tensor_tensor(
                            sq[:, s], tt[:, s], tt[:, s], op=ALU.mult))
                    act(lambda: nc.scalar.activation(a_t[:, s], sq[:, s],
                                                     AF.Exp, scale=-ALPHA))
                    act(lambda: nc.scalar.activation(rho[:, s], tt[:, s],
                                                     AF.Exp,
                                                     scale=float(RHO_SCALE)))
                    # B4 per part so the E4 re-seed never stalls the chain
                    act(lambda: nc.scalar.activation(
                        b4_t[:, s], tt[:, s], AF.Exp,
                        scale=float(2.0 * ALPHA * CENTERS[4])))

                # chains chunk-granular, full g-sweep per chunk — matches the
                # PE consumption order (per-bc-chunk g-ladders)
                for lo, hi in parts:
                    s = slice(hb + lo, hb + hi)
                    sl = slice(lo, hi)
                    for g in range(G):
                        if g in SEED_GS:
                            seed = b0_t if g == 0 else b4_t
                            dve(lambda: nc.vector.tensor_tensor(
                                e_t[h][g][:, sl], a_t[:, s], seed[:, s],
                                op=ALU.mult))
                        else:
                            dve(lambda: nc.vector.tensor_tensor(
                                e_t[h][g][:, sl], e_t[h][g - 1][:, sl],
                                rho[:, s], op=ALU.mult))

            # ---- matmuls ----
            # Per (half, ib, bc) chunk: a full g-ladder of 32 matmuls.  Each
            # ladder consumes one 512-col E chunk per g, which the ACT->DVE
            # chain produces chunk-by-chunk in the same order, so PE never
            # waits cross-chunk.  Bank psum[ot][bc] sees k-tiles in a bank-
            # private order (accumulation order is free).
            for h in range(2):
                for ib_loc in range(2):
                    for bc in range(N_BC):
                        # NB: 256-col matmuls cost the same ~213ns as 512-col
                        # ones (per-matmul LDWEIGHTS/issue floor), so ladders
                        # always run full 512 columns.
                        for g in range(G):
                            kt = h * 16 + ib_loc * 8 + g
                            first = kt == 0
                            last = kt == N_KT - 1
                            base = ib_loc * B_SH + bc * 512
                            rhs = e_t[h][g][:, base:base + 512]
                            # close banks high-ot-first on the stop sweep so
                            # the drain's engine queues line up with close
                            # order
                            ots = range(N_OT - 1, -1, -1) if last \
                                else range(N_OT)
                            for ot in ots:
                                lhsT = w2_all[:, kt * O_SZ + ot * 128:
                                              kt * O_SZ + (ot + 1) * 128]
                                nc.tensor.matmul(
                                    psum[ot][bc][:], lhsT, rhs,
                                    start=first, stop=last)

            # ---- drain: psum -> SBUF bf16 -> DMAs out ----
            # Only ACT and DVE can read PSUM; DMA cannot.  bc0 banks close
            # one g-ladder (~1.7us) before bc1, so their copies + DMAs overlap
            # the final matmul ladder.  Per-(ot,bc) DMAs so each waits on just
            # one copy; sync takes even ot, scalar odd.
            sync_outs = [w2_dmas[-1]]
            dma_eng = {  # (bc, ot) -> issuing queue
                (0, 0): "g", (0, 1): "g", (0, 2): "y", (0, 3): "y",
                (1, 3): "y", (1, 2): "s", (1, 1): "y", (1, 0): "s",
            }
            copy_sc = {0: (0, 1), 1: (3, 1)}   # bc -> ots copied on scalar
            for bc in range(N_BC):
                ot_order = [0, 1, 2, 3] if bc == 0 else [3, 2, 1, 0]
                for ot in ot_order:
                    dst = o_sb[:, (ot * N_BC + bc) * 512:
                               (ot * N_BC + bc + 1) * 512]
                    if ot in copy_sc[bc]:
                        act(lambda: nc.scalar.activation(dst, psum[ot][bc][:],
                                                         AF.Copy))
                    else:
                        dve(lambda: nc.vector.tensor_copy(dst, psum[ot][bc][:]))
                for ot in ot_order:
                    e = dma_eng[(bc, ot)]
                    eng = {"g": nc.gpsimd, "y": nc.sync, "s": nc.scalar}[e]
                    d = eng.dma_start(
                        out_d[ot * 128:(ot + 1) * 128,
                              bc * 512:(bc + 1) * 512],
                        o_sb[:, (ot * N_BC + bc) * 512:
                             (ot * N_BC + bc + 1) * 512],
                    )
                    if e == "y":
                        add_dep_helper(d.ins, sync_outs[-1].ins, sync=False,
                                       reason="sync out order")
                        sync_outs.append(d)
                    elif e == "s":
                        add_dep_helper(d.ins, act_chain[-1].ins, sync=False,
                                       reason="scalar out order")
                        act_chain.append(d)
                    else:
                        add_dep_helper(d.ins, gps_chain[-1].ins, sync=False,
                                       reason="gpsimd out order")
                        gps_chain.append(d)
    nc.compile()
    return nc


def get_nc():
    if "nc" not in _NC_CACHE:
        _NC_CACHE["nc"] = build_nc()
    return _NC_CACHE["nc"]


def prep_inputs(x, weights, coefficients):
    x = np.asarray(x, dtype=np.float32)
    weights = np.asarray(weights, dtype=np.float32)
    coefficients = np.asarray(coefficients, dtype=np.float32)
    # W2T[k=(g,i), o] = coeff[o,i,g] * W[o,i] * exp(-a*c_g^2)
    w2t = (coefficients.astype(np.float64)
           * weights[:, :, None].astype(np.float64)).transpose(2, 1, 0)  # [g,i,o]
    gauss_bias = np.exp(-ALPHA * CENTERS ** 2)  # [G]
    w2t = w2t * gauss_bias[:, None, None]
    # device k-tile order: kt = h*16 + ib_loc*8 + g  (ib_global = 2h + ib_loc)
    w2t = w2t.reshape(G, N_IBLK, 128, O_SZ)  # [g, ib, p, o]
    order = [(g, 2 * h + ib) for h in range(2) for ib in range(2)
             for g in range(G)]
    w2t = np.stack([w2t[g, ib] for g, ib in order], 0)  # [32,128,O]
    w2t = np.ascontiguousarray(w2t.reshape(K, O_SZ)).astype(ml_dtypes.bfloat16)
    xT = np.ascontiguousarray(x.T.astype(ml_dtypes.bfloat16))  # [I, B]
    in_maps = [
        {
            "xt": np.ascontiguousarray(xT[:, c * B_SH:(c + 1) * B_SH]),
            "w2t": w2t,
        }
        for c in range(NCORES)
    ]
    return in_maps


def kernel(x, weights, coefficients):
    nc = get_nc()
    in_maps = prep_inputs(x, weights, coefficients)
    res = run_bass_kernel_spmd(nc, in_maps, core_ids=list(range(NCORES)))
    out = np.empty((B, O_SZ), dtype=np.float32)
    for c in range(NCORES):
        out[c * B_SH:(c + 1) * B_SH, :] = \
            np.asarray(res.results[c]["out_t"], dtype=np.float32).T
    return out
